# revision 3
# baseline (speedup 1.0000x reference)
"""EventSpecificTimingHeads Trainium2 kernel (8 NeuronCores, SPMD), v2.

Shards the E=16 independent per-event attention+MLP heads across 8 cores
(2 events per core). Each core computes logits[e, b, s] for its 2 events
over the full shared feature tensor; the host gathers and transposes to
[B, S, E].

Math per event e (Wo folded into W1 on the host: Wf = W1 @ Wo):
  qkv = x @ Wqkv[e].T + bqkv[e]   (q pre-scaled by 1/sqrt(Dh) via weights)
  per (b, h):  S.T = k q.T (j, i layout);  P.T = exp(S.T)  (shift-free:
  |scores| small so exp is safe)
  ctx.T[32h+d, i] = sum_j v[j, 32h+d] P.T[j, i]   -- M=32, 4-way col-tiled,
  all four heads land contiguously in one psum tile
  l[h, i] = sum_j P.T[j, i]  -- M=1 ones-matmuls into rows {0,32,64,96}
  bcast(l) via 4 tiny K=1 diagonal-tile matmuls -> [128, 512] psum
  ctxn = ctx.T * recip_approx(bcast(l))          (per-head softmax denom)
  h1 = relu(Wf ctxn + c1), Wf = W1 Wo, c1 = W1 (Wo bv + bo) + b1
  logits = w2.T h1                                (b2 added on host)
"""
import sys

if "/opt/trn_rl_repo" not in sys.path:
    sys.path.insert(0, "/opt/trn_rl_repo")

import numpy as np
import ml_dtypes

import concourse.bass as bass
import concourse.bacc as bacc
import concourse.tile as tile
from concourse import mybir
from concourse.bass_utils import run_bass_kernel_spmd

BF16 = mybir.dt.bfloat16
F32 = mybir.dt.float32
AF = mybir.ActivationFunctionType
ALU = mybir.AluOpType

E, D, B, S, H, Dh, H2 = 16, 128, 8, 512, 4, 32, 64
T = B * S            # 4096
EV = 2               # events per core
NCORES = 8

_CACHED_NC = None


def build_nc():
    nc = bacc.Bacc(None, target_bir_lowering=False, debug=False)

    xT_d = nc.declare_dram_parameter("xT", [D, T], BF16, isOutput=False)
    wqkvT_d = nc.declare_dram_parameter("wqkvT", [D, EV, 3, D], BF16, isOutput=False)
    bqk_d = nc.declare_dram_parameter("bqk", [D, EV, 2], F32, isOutput=False)
    wfT_d = nc.declare_dram_parameter("wfT", [D, EV, H2], BF16, isOutput=False)
    c1b_d = nc.declare_dram_parameter("c1b", [H2, EV], F32, isOutput=False)
    w2_d = nc.declare_dram_parameter("w2", [H2, EV], BF16, isOutput=False)
    out_d = nc.declare_dram_parameter("out", [EV, B, S], F32, isOutput=True)

    with tile.TileContext(nc) as tc:
        with (
            tc.tile_pool(name="single", bufs=1) as single,
            tc.tile_pool(name="work", bufs=2) as work,
            tc.tile_pool(name="stp", bufs=2, space="PSUM") as stp,
            tc.tile_pool(name="pvp", bufs=2, space="PSUM") as pvp,
            tc.tile_pool(name="misc", bufs=2, space="PSUM") as misc,
        ):
            # ---- resident SBUF tensors ----
            xT_sb = single.tile([D, T], BF16)
            wqkvT_sb = single.tile([D, EV, 3, D], BF16)
            bqk_sb = single.tile([D, EV, 2], F32)
            wfT_sb = single.tile([D, EV, H2], BF16)
            c1b_sb = single.tile([H2, EV], F32)
            w2_sb = single.tile([H2, EV], BF16)
            qT_sb = single.tile([D, EV, T], BF16)
            kT_sb = single.tile([D, EV, T], BF16)
            # v: [j-in-chunk, tch, ev, h, dh]; tch = 4*b + jc
            v_sb = single.tile([D, 4 * B, EV, H, Dh], BF16)
            ones1 = single.tile([D, 1], BF16)     # lhsT for l row-sums
            ones32 = single.tile([D, Dh], BF16)   # lhsT for l broadcast

            # needed-first weights on sync; bulk xT spread across queues
            nc.sync.dma_start(out=wqkvT_sb[:], in_=wqkvT_d[:])
            nc.sync.dma_start(out=bqk_sb[:], in_=bqk_d[:])
            qs = [nc.scalar, nc.gpsimd]
            for n in range(8):
                qs[n % 2].dma_start(out=xT_sb[:, n * S:(n + 1) * S],
                                    in_=xT_d[:, n * S:(n + 1) * S])
            nc.sync.dma_start(out=wfT_sb[:], in_=wfT_d[:])
            nc.sync.dma_start(out=c1b_sb[:], in_=c1b_d[:])
            nc.sync.dma_start(out=w2_sb[:], in_=w2_d[:])
            nc.gpsimd.memset(ones1[:], 1.0)
            nc.gpsimd.memset(ones32[:], 1.0)

            # ---- q/k projection; bias folded in via tensor_scalar drain
            def proj_chunk(n):
                for ev in range(EV):
                    for qk in range(2):
                        dst = qT_sb if qk == 0 else kT_sb
                        ps = misc.tile([D, S], F32, name="proj_ps", tag="m")
                        nc.tensor.matmul(
                            ps[:],
                            wqkvT_sb[:, ev, qk, :],
                            xT_sb[:, n * S:(n + 1) * S],
                        )
                        nc.vector.tensor_scalar_add(
                            dst[:, ev, n * S:(n + 1) * S],
                            ps[:],
                            bqk_sb[:, ev, qk:qk + 1],
                        )

            for n in range(8):
                proj_chunk(n)

            def project_v(b):
                # both events at once: rhs [128, 2*128], two t-chunks per psum
                for half in range(2):
                    psv = pvp.tile([D, S], F32, name="vproj_ps", tag="pv")
                    for c2 in range(2):
                        tch = 4 * b + 2 * half + c2
                        nc.tensor.matmul(
                            psv[:, c2 * 256:(c2 + 1) * 256],
                            xT_sb[:, tch * D:(tch + 1) * D],
                            wqkvT_sb[:, :, 2, :],
                        )
                    # psum col layout (c2, ev, h, dh) -> v_sb[:, tch, ev, h, dh]
                    t0c = 4 * b + 2 * half
                    nc.vector.tensor_copy(
                        v_sb[:, t0c:t0c + 2, :, :, :],
                        psv[:].rearrange("p (c e h d) -> p c e h d",
                                         c=2, e=EV, h=H),
                    )

            # ---- main per-(event, batch) pipeline ----
            for ev in range(EV):
                for b in range(B):
                    t0 = b * S
                    if ev == 0:
                        project_v(b)
                    # QK^T (transposed orientation) + exp; pt in SBUF bf16
                    pt = work.tile([D, 4, H, S], BF16, name="pt")
                    ctx_ps = pvp.tile([D, S], F32, name="ctx_ps", tag="pv")
                    lrows = pvp.tile([D, S], F32, name="lrows", tag="pv")
                    for jc in range(4):
                        sts = [stp.tile([D, 2, S], F32, name=f"st{hp}", tag="st")
                               for hp in range(2)]
                        for h in range(H):
                            nc.tensor.matmul(
                                sts[h // 2][:, h % 2, :],
                                kT_sb[32 * h:32 * h + 32, ev,
                                      t0 + jc * D:t0 + (jc + 1) * D],
                                qT_sb[32 * h:32 * h + 32, ev, t0:t0 + S],
                                tile_position=(32 * h, 0),
                            )
                        for hp in range(2):
                            nc.scalar.activation(
                                pt[:, jc, 2 * hp:2 * hp + 2, :],
                                sts[hp][:], AF.Exp,
                            )
                        # PV: M=32 per head, 4-way col tiling; ctx lands
                        # contiguously as [4h*32, i] in one psum bank
                        tch = 4 * b + jc
                        for h in range(H):
                            nc.tensor.matmul(
                                ctx_ps[32 * h:32 * h + 32, :],
                                v_sb[:, tch, ev, h, :],
                                pt[:, jc, h, :],
                                start=(jc == 0),
                                stop=(jc == 3),
                                tile_position=(0, 32 * h),
                            )
                        # l row-sums: M=1 ones-matmuls into rows {0,32,64,96}
                        for h in range(H):
                            nc.tensor.matmul(
                                lrows[32 * h:32 * h + 1, :],
                                ones1[:, 0:1],
                                pt[:, jc, h, :],
                                start=(jc == 0),
                                stop=(jc == 3),
                                tile_position=(0, 32 * h),
                            )
                    # broadcast l to [128, 512]: K=1 diagonal-tile matmuls
                    lsb = work.tile([D, S], BF16, name="lsb")
                    nc.vector.tensor_copy(lsb[:], lrows[:])
                    bca = misc.tile([D, S], F32, name="bca", tag="m")
                    for h in range(H):
                        nc.tensor.matmul(
                            bca[32 * h:32 * h + 32, :],
                            ones32[32 * h:32 * h + 1, :],
                            lsb[32 * h:32 * h + 1, :],
                            tile_position=(32 * h, 32 * h),
                        )
                    linvb = work.tile([D, S], F32, name="linvb")
                    nc.vector.reciprocal_approx_fast(out=linvb[:], in_=bca[:])
                    ctxn = work.tile([D, S], BF16, name="ctxn")
                    nc.vector.tensor_tensor(ctxn[:], ctx_ps[:], linvb[:],
                                            ALU.mult)
                    # fused W1@Wo + relu(g + c1)
                    gp = misc.tile([H2, S], F32, name="gp", tag="m")
                    nc.tensor.matmul(gp[:], wfT_sb[:, ev, :], ctxn[:])
                    h1 = work.tile([H2, S], BF16, name="h1")
                    nc.vector.tensor_scalar(
                        h1[:],
                        gp[:],
                        c1b_sb[:, ev:ev + 1],
                        0.0,
                        ALU.add,
                        ALU.max,
                    )
                    # W2 (K=64, M=1) and store
                    lgp = misc.tile([1, S], F32, name="lgp", tag="m")
                    nc.tensor.matmul(lgp[:], w2_sb[:, ev:ev + 1], h1[:])
                    lg_sb = work.tile([1, S], F32, name="lg_sb")
                    nc.vector.tensor_copy(lg_sb[:], lgp[:])
                    nc.sync.dma_start(out=out_d[ev, b, :], in_=lg_sb[0:1, :])

    nc.compile()
    return nc


def _prep_inputs(lstm_features, Wqkv, bqkv, Wo, bo, W1, b1, W2, b2):
    """Host-side per-core input prep (numpy, fp32 -> bf16 where PE-facing)."""
    bf = ml_dtypes.bfloat16
    x = np.asarray(lstm_features, np.float32).reshape(T, D)
    xT = np.ascontiguousarray(x.T).astype(bf)
    scale = 1.0 / np.sqrt(np.float32(Dh))

    in_maps = []
    for c in range(NCORES):
        evs = [2 * c, 2 * c + 1]
        wqkvT = np.zeros((D, EV, 3, D), np.float32)
        bqk = np.zeros((D, EV, 2), np.float32)
        wfT = np.zeros((D, EV, H2), np.float32)
        c1b = np.zeros((H2, EV), np.float32)
        w2 = np.zeros((H2, EV), np.float32)
        for i, e in enumerate(evs):
            Wq = Wqkv[e, 0:D, :] * scale
            Wk = Wqkv[e, D:2 * D, :]
            Wv = Wqkv[e, 2 * D:3 * D, :]
            wqkvT[:, i, 0, :] = Wq.T
            wqkvT[:, i, 1, :] = Wk.T
            wqkvT[:, i, 2, :] = Wv.T
            bqk[:, i, 0] = bqkv[e, 0:D] * scale
            bqk[:, i, 1] = bqkv[e, D:2 * D]
            bv = bqkv[e, 2 * D:3 * D]
            bo_eff = Wo[e] @ bv + bo[e]
            Wf = W1[e] @ Wo[e]          # fold Wo into W1
            wfT[:, i, :] = Wf.T
            c1b[:, i] = W1[e] @ bo_eff + b1[e]
            w2[:, i] = W2[e, 0, :]
        in_maps.append({
            "xT": xT,
            "wqkvT": wqkvT.astype(bf),
            "bqk": bqk,
            "wfT": wfT.astype(bf),
            "c1b": c1b,
            "w2": w2.astype(bf),
        })
    return in_maps


def kernel(lstm_features, Wqkv, bqkv, Wo, bo, W1, b1, W2, b2, _trace=False):
    global _CACHED_NC
    args = [np.asarray(a, np.float32) for a in
            (lstm_features, Wqkv, bqkv, Wo, bo, W1, b1, W2, b2)]
    in_maps = _prep_inputs(*args)
    if _CACHED_NC is None:
        _CACHED_NC = build_nc()
    res = run_bass_kernel_spmd(
        _CACHED_NC, in_maps, list(range(NCORES)), trace=_trace
    )
    logits = np.concatenate(
        [np.asarray(res.results[c]["out"], np.float32) for c in range(NCORES)],
        axis=0,
    )  # [16, 8, 512]
    logits += np.asarray(args[8], np.float32).reshape(E, 1, 1)  # + b2 on host
    out = np.ascontiguousarray(logits.transpose(1, 2, 0))  # [B, S, E]
    if _trace:
        return out, res
    return out


# revision 5
# speedup vs baseline: 1.0009x; 1.0009x over previous
"""EventSpecificTimingHeads Trainium2 kernel (8 NeuronCores, SPMD), v3.

Shards the E=16 independent per-event attention+MLP heads across 8 cores
(2 events per core). Each core computes logits[e, b, s] for its 2 events
over the full shared feature tensor; the host gathers and transposes to
[B, S, E].

Math per event e (Wo folded into W1 on the host: Wf = W1 @ Wo):
  qkv = x @ Wqkv[e].T + bqkv[e]   (q pre-scaled by 1/sqrt(Dh) via weights)
  per (b, h):  S.T = k q.T (j, i layout);  P.T = exp(S.T)  (shift-free)
  PV with M=64 augmented lhsT [v_h | 1 | 0*31]: each stream yields
  ctx_h (32 rows) + l_h (row 32) + zeros; heads packed two per psum bank
  at 64-row offsets (pv2[:, t, :], t = head pair).
  linv = reciprocal_approx_fast(pv2)  (rows 32/96 hold 1/l; rest unused)
  bc2 = broadcast linv rows via K=1 matmuls -> [row-block, 512] per head
  ctxn = pv2 * bc2  (per-head softmax normalize; l/zero rows are junk
  but finite -- the permuted Wf has zero weights there)
  gp = WfA_perm ctxnA + WfB_perm ctxnB   (Wf = W1 Wo, row-permuted)
  h1 = relu(gp + c1), c1 = W1 (Wo bv + bo) + b1
  logits = w2.T h1                       (b2 added on host)
"""
import sys

if "/opt/trn_rl_repo" not in sys.path:
    sys.path.insert(0, "/opt/trn_rl_repo")

import numpy as np
import ml_dtypes

import concourse.bass as bass
import concourse.bacc as bacc
import concourse.tile as tile
from concourse import mybir
from concourse.bass_utils import run_bass_kernel_spmd

BF16 = mybir.dt.bfloat16
F32 = mybir.dt.float32
AF = mybir.ActivationFunctionType
ALU = mybir.AluOpType

E, D, B, S, H, Dh, H2 = 16, 128, 8, 512, 4, 32, 64
T = B * S            # 4096
EV = 2               # events per core
NCORES = 8

_CACHED_NC = None


def build_nc():
    nc = bacc.Bacc(None, target_bir_lowering=False, debug=False)

    xT_d = nc.declare_dram_parameter("xT", [D, T], BF16, isOutput=False)
    wqkvT_d = nc.declare_dram_parameter("wqkvT", [D, EV, 3, D], BF16, isOutput=False)
    bqk_d = nc.declare_dram_parameter("bqk", [D, EV, 2], F32, isOutput=False)
    wfTp_d = nc.declare_dram_parameter("wfTp", [D, EV, 2, H2], BF16, isOutput=False)
    c1b_d = nc.declare_dram_parameter("c1b", [H2, EV], F32, isOutput=False)
    w2_d = nc.declare_dram_parameter("w2", [H2, EV], BF16, isOutput=False)
    out_d = nc.declare_dram_parameter("out", [EV, B, S], F32, isOutput=True)

    with tile.TileContext(nc) as tc:
        with (
            tc.tile_pool(name="single", bufs=1) as single,
            tc.tile_pool(name="work", bufs=2) as work,
            tc.tile_pool(name="stp", bufs=2, space="PSUM") as stp,
            tc.tile_pool(name="big", bufs=2, space="PSUM") as big,
        ):
            # ---- resident SBUF tensors ----
            xT_sb = single.tile([D, T], BF16)
            wqkvT_sb = single.tile([D, EV, 3, D], BF16)
            bqk_sb = single.tile([D, EV, 2], F32)
            wfTp_sb = single.tile([D, EV, 2, H2], BF16)
            c1b_sb = single.tile([H2, EV], F32)
            w2_sb = single.tile([H2, EV], BF16)
            qT_sb = single.tile([D, EV, T], BF16)
            kT_sb = single.tile([D, EV, T], BF16)
            # augmented v: [j-in-chunk, tch, ev, h, 64]; cols = [v | 1 | 0*31]
            v_sb = single.tile([D, 4 * B, EV, H, 2 * Dh], BF16)
            ones64 = single.tile([D, H2], F32)   # lhsT for linv broadcast

            nc.sync.dma_start(out=wqkvT_sb[:], in_=wqkvT_d[:])
            nc.sync.dma_start(out=bqk_sb[:], in_=bqk_d[:])
            qs = [nc.scalar, nc.gpsimd]
            for n in range(8):
                qs[n % 2].dma_start(out=xT_sb[:, n * S:(n + 1) * S],
                                    in_=xT_d[:, n * S:(n + 1) * S])
            nc.sync.dma_start(out=wfTp_sb[:], in_=wfTp_d[:])
            nc.sync.dma_start(out=c1b_sb[:], in_=c1b_d[:])
            nc.sync.dma_start(out=w2_sb[:], in_=w2_d[:])
            nc.gpsimd.memset(ones64[:], 1.0)
            nc.gpsimd.memset(v_sb[:, :, :, :, Dh:Dh + 1], 1.0)
            nc.gpsimd.memset(v_sb[:, :, :, :, Dh + 1:], 0.0)

            # ---- q/k projection; bias folded in via tensor_scalar drain
            def proj_chunk(n):
                for ev in range(EV):
                    for qk in range(2):
                        dst = qT_sb if qk == 0 else kT_sb
                        ps = big.tile([D, S], F32, name="proj_ps", tag="m")
                        nc.tensor.matmul(
                            ps[:],
                            wqkvT_sb[:, ev, qk, :],
                            xT_sb[:, n * S:(n + 1) * S],
                        )
                        nc.any.tensor_scalar_add(
                            dst[:, ev, n * S:(n + 1) * S],
                            ps[:],
                            bqk_sb[:, ev, qk:qk + 1],
                        )

            for n in range(8):
                proj_chunk(n)

            def project_v(b):
                # both events at once: rhs [128, 2*128], two t-chunks per psum
                for half in range(2):
                    psv = big.tile([D, S], F32, name="vproj_ps", tag="m")
                    for c2 in range(2):
                        tch = 4 * b + 2 * half + c2
                        nc.tensor.matmul(
                            psv[:, c2 * 256:(c2 + 1) * 256],
                            xT_sb[:, tch * D:(tch + 1) * D],
                            wqkvT_sb[:, :, 2, :],
                        )
                    t0c = 4 * b + 2 * half
                    nc.any.tensor_copy(
                        v_sb[:, t0c:t0c + 2, :, :, 0:Dh],
                        psv[:].rearrange("p (c e h d) -> p c e h d",
                                         c=2, e=EV, h=H),
                    )

            # ---- main per-(event, batch) pipeline ----
            for ev in range(EV):
                for b in range(B):
                    t0 = b * S
                    if ev == 0:
                        project_v(b)
                    pt = work.tile([D, 4, H, S], BF16, name="pt")
                    # pv2[:, t, :]: rows 0:33 = ctx/l head 2t, 64:97 head 2t+1
                    pv2 = big.tile([D, 2, S], F32, name="pv2", tag="m")
                    for jc in range(4):
                        sts = [stp.tile([D, 2, S], F32, name=f"st{hp}", tag="st")
                               for hp in range(2)]
                        for h in range(H):
                            nc.tensor.matmul(
                                sts[h // 2][:, h % 2, :],
                                kT_sb[32 * h:32 * h + 32, ev,
                                      t0 + jc * D:t0 + (jc + 1) * D],
                                qT_sb[32 * h:32 * h + 32, ev, t0:t0 + S],
                                tile_position=(32 * h, 0),
                            )
                        for hp in range(2):
                            nc.scalar.activation(
                                pt[:, jc, 2 * hp:2 * hp + 2, :],
                                sts[hp][:], AF.Exp,
                            )
                        # PV, M=64 augmented: ctx+l+zeros per head, two
                        # heads per bank at 64-row offsets
                        tch = 4 * b + jc
                        for h in range(H):
                            nc.tensor.matmul(
                                pv2[(h % 2) * 64:(h % 2) * 64 + 64, h // 2, :],
                                v_sb[:, tch, ev, h, :],
                                pt[:, jc, h, :],
                                start=(jc == 0),
                                stop=(jc == 3),
                                tile_position=(0, (h % 2) * 64),
                            )
                    # 1/l lands on rows 32 (head 2t) and 96 (head 2t+1)
                    linv = work.tile([D, 2, S], F32, name="linv")
                    nc.vector.reciprocal_approx_fast(out=linv[:], in_=pv2[:])
                    # broadcast linv rows to 64-row blocks: K=1 matmuls
                    bc2 = big.tile([D, 2, S], F32, name="bc2", tag="m")
                    for t in range(2):
                        nc.tensor.matmul(
                            bc2[0:64, t, :],
                            ones64[32:33, :],
                            linv[32:33, t, :],
                            tile_position=(32, 0),
                        )
                        nc.tensor.matmul(
                            bc2[64:128, t, :],
                            ones64[96:97, :],
                            linv[96:97, t, :],
                            tile_position=(96, 64),
                        )
                    bc2_sb = work.tile([D, 2, S], F32, name="bc2_sb")
                    nc.vector.tensor_copy(bc2_sb[:], bc2[:])
                    ctxn = work.tile([D, 2, S], BF16, name="ctxn")
                    nc.vector.tensor_tensor(ctxn[:], pv2[:], bc2_sb[:],
                                            ALU.mult)
                    # permuted fused W1@Wo over both head-pair tiles
                    gp = big.tile([H2, S], F32, name="gp", tag="m")
                    for t in range(2):
                        nc.tensor.matmul(gp[:], wfTp_sb[:, ev, t, :],
                                         ctxn[:, t, :],
                                         start=(t == 0), stop=(t == 1))
                    h1 = work.tile([H2, S], BF16, name="h1")
                    nc.vector.tensor_scalar(
                        h1[:],
                        gp[:],
                        c1b_sb[:, ev:ev + 1],
                        0.0,
                        ALU.add,
                        ALU.max,
                    )
                    lgp = big.tile([1, S], F32, name="lgp", tag="m")
                    nc.tensor.matmul(lgp[:], w2_sb[:, ev:ev + 1], h1[:])
                    lg_sb = work.tile([1, S], F32, name="lg_sb")
                    nc.vector.tensor_copy(lg_sb[:], lgp[:])
                    nc.sync.dma_start(out=out_d[ev, b, :], in_=lg_sb[0:1, :])

    nc.compile()
    return nc


def _prep_inputs(lstm_features, Wqkv, bqkv, Wo, bo, W1, b1, W2, b2):
    """Host-side per-core input prep (numpy, fp32 -> bf16 where PE-facing)."""
    bf = ml_dtypes.bfloat16
    x = np.asarray(lstm_features, np.float32).reshape(T, D)
    xT = np.ascontiguousarray(x.T).astype(bf)
    scale = 1.0 / np.sqrt(np.float32(Dh))

    in_maps = []
    for c in range(NCORES):
        evs = [2 * c, 2 * c + 1]
        wqkvT = np.zeros((D, EV, 3, D), np.float32)
        bqk = np.zeros((D, EV, 2), np.float32)
        wfTp = np.zeros((D, EV, 2, H2), np.float32)
        c1b = np.zeros((H2, EV), np.float32)
        w2 = np.zeros((H2, EV), np.float32)
        for i, e in enumerate(evs):
            Wq = Wqkv[e, 0:D, :] * scale
            Wk = Wqkv[e, D:2 * D, :]
            Wv = Wqkv[e, 2 * D:3 * D, :]
            wqkvT[:, i, 0, :] = Wq.T
            wqkvT[:, i, 1, :] = Wk.T
            wqkvT[:, i, 2, :] = Wv.T
            bqk[:, i, 0] = bqkv[e, 0:D] * scale
            bqk[:, i, 1] = bqkv[e, D:2 * D]
            bv = bqkv[e, 2 * D:3 * D]
            bo_eff = Wo[e] @ bv + bo[e]
            Wf = W1[e] @ Wo[e]          # fold Wo into W1: [H2, D]
            # permuted lhsT rows matching ctxn layout: tile t row r:
            #   r in [0,32)  -> head 2t   dims -> Wf cols 32*(2t)+r
            #   r in [64,96) -> head 2t+1 dims -> Wf cols 32*(2t+1)+(r-64)
            for t in range(2):
                wfTp[0:Dh, i, t, :] = Wf[:, 64 * t:64 * t + Dh].T
                wfTp[64:64 + Dh, i, t, :] = Wf[:, 64 * t + Dh:64 * t + 64].T
            c1b[:, i] = W1[e] @ bo_eff + b1[e]
            w2[:, i] = W2[e, 0, :]
        in_maps.append({
            "xT": xT,
            "wqkvT": wqkvT.astype(bf),
            "bqk": bqk,
            "wfTp": wfTp.astype(bf),
            "c1b": c1b,
            "w2": w2.astype(bf),
        })
    return in_maps


def kernel(lstm_features, Wqkv, bqkv, Wo, bo, W1, b1, W2, b2, _trace=False):
    global _CACHED_NC
    args = [np.asarray(a, np.float32) for a in
            (lstm_features, Wqkv, bqkv, Wo, bo, W1, b1, W2, b2)]
    in_maps = _prep_inputs(*args)
    if _CACHED_NC is None:
        _CACHED_NC = build_nc()
    res = run_bass_kernel_spmd(
        _CACHED_NC, in_maps, list(range(NCORES)), trace=_trace
    )
    logits = np.concatenate(
        [np.asarray(res.results[c]["out"], np.float32) for c in range(NCORES)],
        axis=0,
    )  # [16, 8, 512]
    logits += np.asarray(args[8], np.float32).reshape(E, 1, 1)  # + b2 on host
    out = np.ascontiguousarray(logits.transpose(1, 2, 0))  # [B, S, E]
    if _trace:
        return out, res
    return out


# revision 7
# speedup vs baseline: 1.0485x; 1.0475x over previous
"""EventSpecificTimingHeads Trainium2 kernel (8 NeuronCores, SPMD), v4.

Shards the E=16 independent per-event attention+MLP heads across 8 cores
(2 events per core). Each core computes logits[e, b, s] for its 2 events
over the full shared feature tensor; the host gathers and transposes to
[B, S, E].

Math per event e (Wo folded into W1 on the host: Wf = W1 @ Wo):
  qkv = x @ Wqkv[e].T + bqkv[e]   (q pre-scaled by 1/sqrt(Dh) via weights)
  per (b, h):  S.T = k q.T (j, i layout);  P.T = exp(S.T)  (shift-free)
  PV with M=64 augmented lhsT [v_h | 1 | 0*31]: each stream yields
  ctx_h (32 rows) + l_h (row 32) + zeros; heads packed two per psum bank
  at 64-row offsets (pv2[:, t, :], t = head pair).
  linv = reciprocal_approx_fast(pv2) -> bf16 (rows 32/96 hold 1/l)
  bc2 = broadcast linv rows via K=1 bf16 matmuls -> 64-row blocks
  ctxn = pv2 * bc2  (per-head softmax normalize; l/zero rows are junk
  but finite -- the permuted Wf has zero weights there)
  gp = WfA_perm ctxnA + WfB_perm ctxnB   (Wf = W1 Wo, row-permuted)
  h1 = relu(gp + c1), c1 = W1 (Wo bv + bo) + b1
  logits = w2.T h1                       (b2 added on host)

The emission is software-pipelined: iteration i+1's first QK/exp block
is emitted before iteration i's normalize/MLP tail so the scalar engine
(the exp bottleneck) never starves behind the tail's dependency chain.
"""
import sys

if "/opt/trn_rl_repo" not in sys.path:
    sys.path.insert(0, "/opt/trn_rl_repo")

import numpy as np
import ml_dtypes

import concourse.bass as bass
import concourse.bacc as bacc
import concourse.tile as tile
from concourse import mybir
from concourse.bass_utils import run_bass_kernel_spmd
from concourse.dve_ops import RECIPROCAL_APPROX_FAST, RECIP_APPROX_FAST_CONSTS

BF16 = mybir.dt.bfloat16
F32 = mybir.dt.float32
AF = mybir.ActivationFunctionType
ALU = mybir.AluOpType

E, D, B, S, H, Dh, H2 = 16, 128, 8, 512, 4, 32, 64
T = B * S            # 4096
EV = 2               # events per core
NCORES = 8

_CACHED_NC = None


def build_nc():
    nc = bacc.Bacc(None, target_bir_lowering=False, debug=False)

    xT_d = nc.declare_dram_parameter("xT", [D, T], BF16, isOutput=False)
    wqkvT_d = nc.declare_dram_parameter("wqkvT", [D, EV, 3, D], BF16, isOutput=False)
    bqk_d = nc.declare_dram_parameter("bqk", [D, EV, 2], F32, isOutput=False)
    wfTp_d = nc.declare_dram_parameter("wfTp", [D, EV, 2, H2], BF16, isOutput=False)
    c1b_d = nc.declare_dram_parameter("c1b", [H2, EV], F32, isOutput=False)
    w2_d = nc.declare_dram_parameter("w2", [H2, EV], BF16, isOutput=False)
    out_d = nc.declare_dram_parameter("out", [EV, B, S], F32, isOutput=True)

    with tile.TileContext(nc) as tc:
        with (
            tc.tile_pool(name="single", bufs=1) as single,
            tc.tile_pool(name="work", bufs=2) as work,
            tc.tile_pool(name="stp", bufs=2, space="PSUM") as stp,
            tc.tile_pool(name="pvp", bufs=1, space="PSUM") as pvp,
            tc.tile_pool(name="mp", bufs=1, space="PSUM") as mp,
        ):
            # ---- resident SBUF tensors ----
            xT_sb = single.tile([D, T], BF16)
            wqkvT_sb = single.tile([D, EV, 3, D], BF16)
            bqk_sb = single.tile([D, EV, 2], F32)
            wfTp_sb = single.tile([D, EV, 2, H2], BF16)
            c1b_sb = single.tile([H2, EV], F32)
            w2_sb = single.tile([H2, EV], BF16)
            qT_sb = single.tile([D, EV, T], BF16)
            kT_sb = single.tile([D, EV, T], BF16)
            # augmented v: [j-in-chunk, tch, ev, h, 64]; cols = [v | 1 | 0*31]
            v_sb = single.tile([D, 4 * B, EV, H, 2 * Dh], BF16)
            ones64 = single.tile([D, H2], BF16)   # lhsT for linv broadcast

            nc.sync.dma_start(out=wqkvT_sb[:], in_=wqkvT_d[:])
            nc.sync.dma_start(out=bqk_sb[:], in_=bqk_d[:])
            qs = [nc.scalar, nc.gpsimd]
            for n in range(8):
                qs[n % 2].dma_start(out=xT_sb[:, n * S:(n + 1) * S],
                                    in_=xT_d[:, n * S:(n + 1) * S])
            nc.sync.dma_start(out=wfTp_sb[:], in_=wfTp_d[:])
            nc.sync.dma_start(out=c1b_sb[:], in_=c1b_d[:])
            nc.sync.dma_start(out=w2_sb[:], in_=w2_d[:])
            nc.gpsimd.memset(ones64[:], 1.0)
            nc.gpsimd.memset(v_sb[:, :, :, :, Dh:Dh + 1], 1.0)
            nc.gpsimd.memset(v_sb[:, :, :, :, Dh + 1:], 0.0)

            # ---- q/k projection for token chunk n (feeds batch b=n) ----
            def proj_chunk(n):
                for ev in range(EV):
                    for qk in range(2):
                        dst = qT_sb if qk == 0 else kT_sb
                        ps = stp.tile([D, S], F32, name="proj_ps", tag="st")
                        nc.tensor.matmul(
                            ps[:],
                            wqkvT_sb[:, ev, qk, :],
                            xT_sb[:, n * S:(n + 1) * S],
                        )
                        nc.any.tensor_scalar_add(
                            dst[:, ev, n * S:(n + 1) * S],
                            ps[:],
                            bqk_sb[:, ev, qk:qk + 1],
                        )

            def project_v(b):
                # both events at once: rhs [128, 2*128], two t-chunks per psum
                for half in range(2):
                    psv = pvp.tile([D, S], F32, name="vproj_ps", tag="pv")
                    for c2 in range(2):
                        tch = 4 * b + 2 * half + c2
                        nc.tensor.matmul(
                            psv[:, c2 * 256:(c2 + 1) * 256],
                            xT_sb[:, tch * D:(tch + 1) * D],
                            wqkvT_sb[:, :, 2, :],
                        )
                    t0c = 4 * b + 2 * half
                    nc.any.tensor_copy(
                        v_sb[:, t0c:t0c + 2, :, :, 0:Dh],
                        psv[:].rearrange("p (c e h d) -> p c e h d",
                                         c=2, e=EV, h=H),
                    )

            def emit_qk_exp(ev, b, pt, jc):
                t0 = b * S
                sts = [stp.tile([D, 2, S], F32, name=f"st{hp}", tag="st")
                       for hp in range(2)]
                for h in range(H):
                    nc.tensor.matmul(
                        sts[h // 2][:, h % 2, :],
                        kT_sb[32 * h:32 * h + 32, ev,
                              t0 + jc * D:t0 + (jc + 1) * D],
                        qT_sb[32 * h:32 * h + 32, ev, t0:t0 + S],
                        tile_position=(32 * h, 0),
                    )
                for hp in range(2):
                    nc.scalar.activation(
                        pt[:, jc, 2 * hp:2 * hp + 2, :], sts[hp][:], AF.Exp,
                    )

            def emit_pv(ev, b, pt, pv2, jc):
                tch = 4 * b + jc
                for h in range(H):
                    nc.tensor.matmul(
                        pv2[(h % 2) * 64:(h % 2) * 64 + 64, h // 2, :],
                        v_sb[:, tch, ev, h, :],
                        pt[:, jc, h, :],
                        start=(jc == 0),
                        stop=(jc == 3),
                        tile_position=(0, (h % 2) * 64),
                    )

            def emit_tail(ev, b, pv2):
                # 1/l lands on rows 32 (head 2t) and 96 (head 2t+1)
                linv = work.tile([D, 2, S], BF16, name="linv")
                c = RECIP_APPROX_FAST_CONSTS
                nc.vector._custom_dve(
                    RECIPROCAL_APPROX_FAST, out=linv[:], in0=pv2[:],
                    s0=c["s0"], s1=c["s1"], imm2=c["imm2"],
                )
                # broadcast linv rows to 64-row blocks: K=1 bf16 matmuls
                bc2 = mp.tile([D, 2, S], F32, name="bc2", tag="m")
                for t in range(2):
                    nc.tensor.matmul(
                        bc2[0:64, t, :],
                        ones64[32:33, :],
                        linv[32:33, t, :],
                        tile_position=(32, 0),
                    )
                    nc.tensor.matmul(
                        bc2[64:128, t, :],
                        ones64[96:97, :],
                        linv[96:97, t, :],
                        tile_position=(96, 64),
                    )
                bc2_sb = work.tile([D, 2, S], BF16, name="bc2_sb")
                nc.vector.tensor_copy(bc2_sb[:], bc2[:])
                ctxn = work.tile([D, 2, S], BF16, name="ctxn")
                nc.vector.tensor_tensor(ctxn[:], pv2[:], bc2_sb[:], ALU.mult)
                # permuted fused W1@Wo over both head-pair tiles
                gp = mp.tile([H2, S], F32, name="gp", tag="m")
                for t in range(2):
                    nc.tensor.matmul(gp[:], wfTp_sb[:, ev, t, :],
                                     ctxn[:, t, :],
                                     start=(t == 0), stop=(t == 1))
                h1 = work.tile([H2, S], BF16, name="h1")
                nc.vector.tensor_scalar(
                    h1[:], gp[:], c1b_sb[:, ev:ev + 1], 0.0, ALU.add, ALU.max,
                )
                lgp = mp.tile([1, S], F32, name="lgp", tag="m")
                nc.tensor.matmul(lgp[:], w2_sb[:, ev:ev + 1], h1[:])
                lg_sb = work.tile([1, S], F32, name="lg_sb")
                nc.vector.tensor_copy(lg_sb[:], lgp[:])
                nc.sync.dma_start(out=out_d[ev, b, :], in_=lg_sb[0:1, :])

            # ---- software-pipelined main loop ----
            proj_chunk(0)
            proj_chunk(1)
            prev = None
            for ev in range(EV):
                for b in range(B):
                    pt = work.tile([D, 4, H, S], BF16, name="pt")
                    emit_qk_exp(ev, b, pt, 0)
                    if ev == 0 and b + 2 < B:
                        proj_chunk(b + 2)
                    if ev == 0:
                        project_v(b)
                    if prev is not None:
                        emit_tail(*prev)
                    pv2 = pvp.tile([D, 2, S], F32, name="pv2", tag="pv")
                    emit_pv(ev, b, pt, pv2, 0)
                    for jc in range(1, 4):
                        emit_qk_exp(ev, b, pt, jc)
                        emit_pv(ev, b, pt, pv2, jc)
                    prev = (ev, b, pv2)
            emit_tail(*prev)

    nc.compile()
    return nc


def _prep_inputs(lstm_features, Wqkv, bqkv, Wo, bo, W1, b1, W2, b2):
    """Host-side per-core input prep (numpy, fp32 -> bf16 where PE-facing)."""
    bf = ml_dtypes.bfloat16
    x = np.asarray(lstm_features, np.float32).reshape(T, D)
    xT = np.ascontiguousarray(x.T).astype(bf)
    scale = 1.0 / np.sqrt(np.float32(Dh))

    in_maps = []
    for c in range(NCORES):
        evs = [2 * c, 2 * c + 1]
        wqkvT = np.zeros((D, EV, 3, D), np.float32)
        bqk = np.zeros((D, EV, 2), np.float32)
        wfTp = np.zeros((D, EV, 2, H2), np.float32)
        c1b = np.zeros((H2, EV), np.float32)
        w2 = np.zeros((H2, EV), np.float32)
        for i, e in enumerate(evs):
            Wq = Wqkv[e, 0:D, :] * scale
            Wk = Wqkv[e, D:2 * D, :]
            Wv = Wqkv[e, 2 * D:3 * D, :]
            wqkvT[:, i, 0, :] = Wq.T
            wqkvT[:, i, 1, :] = Wk.T
            wqkvT[:, i, 2, :] = Wv.T
            bqk[:, i, 0] = bqkv[e, 0:D] * scale
            bqk[:, i, 1] = bqkv[e, D:2 * D]
            bv = bqkv[e, 2 * D:3 * D]
            bo_eff = Wo[e] @ bv + bo[e]
            Wf = W1[e] @ Wo[e]          # fold Wo into W1: [H2, D]
            # permuted lhsT rows matching ctxn layout: tile t row r:
            #   r in [0,32)  -> head 2t   dims -> Wf cols 64t + r
            #   r in [64,96) -> head 2t+1 dims -> Wf cols 64t + 32 + (r-64)
            for t in range(2):
                wfTp[0:Dh, i, t, :] = Wf[:, 64 * t:64 * t + Dh].T
                wfTp[64:64 + Dh, i, t, :] = Wf[:, 64 * t + Dh:64 * t + 64].T
            c1b[:, i] = W1[e] @ bo_eff + b1[e]
            w2[:, i] = W2[e, 0, :]
        in_maps.append({
            "xT": xT,
            "wqkvT": wqkvT.astype(bf),
            "bqk": bqk,
            "wfTp": wfTp.astype(bf),
            "c1b": c1b,
            "w2": w2.astype(bf),
        })
    return in_maps


def kernel(lstm_features, Wqkv, bqkv, Wo, bo, W1, b1, W2, b2, _trace=False):
    global _CACHED_NC
    args = [np.asarray(a, np.float32) for a in
            (lstm_features, Wqkv, bqkv, Wo, bo, W1, b1, W2, b2)]
    in_maps = _prep_inputs(*args)
    if _CACHED_NC is None:
        _CACHED_NC = build_nc()
    res = run_bass_kernel_spmd(
        _CACHED_NC, in_maps, list(range(NCORES)), trace=_trace
    )
    logits = np.concatenate(
        [np.asarray(res.results[c]["out"], np.float32) for c in range(NCORES)],
        axis=0,
    )  # [16, 8, 512]
    logits += np.asarray(args[8], np.float32).reshape(E, 1, 1)  # + b2 on host
    out = np.ascontiguousarray(logits.transpose(1, 2, 0))  # [B, S, E]
    if _trace:
        return out, res
    return out
